# revision 4
# baseline (speedup 1.0000x reference)
"""FFNN-Transducer joint-lattice kernel for 8 Trainium2 NeuronCores.

Data-parallel over batch B=8 (one sample per core). The device computes the
dense T x (U+1) joint lattice:
    out[t,u,:] = tanh(enc_proj[t,:] + pred_bias[u,:]) @ jw2
where enc_proj = enc @ jw1[:E] is computed on-device and pred_bias[u,:]
(= pred @ jw1[E:] + jb1) comes from the tiny prediction network (done on host,
<0.3% of total FLOPs). jb2-add and the ragged masking are host epilogues.

Device pipeline per core (t-major, 1000 t-steps):
  PE:  enc_projT prologue; per 5-t chunk one "selection" matmul that
       materializes A[j,(t,u)] = enc_proj[t,j] + bias[u,j] in PSUM
       (lhsT = [20 enc rows ; 101 bias rows], rhs = 0/1 selection matrix);
       per t one [128x101] x [128x88] joint matmul into PSUM.
  ACT: batched tanh PSUM->SBUF, 1010 elems/instr.
  DVE: PSUM->SBUF staging evacuation (the only legal PSUM->DRAM route).
  DMA: 1.42 MB contiguous stores (40 t-steps each).
"""

import os
import sys

for _p in ("/opt/trn_rl_repo", "/root/.axon_site/_ro/trn_rl_repo"):
    if os.path.isdir(_p) and _p not in sys.path:
        sys.path.append(_p)

import numpy as np

import concourse.bass as bass
import concourse.tile as tile
from concourse import bacc, mybir
from concourse.bass_utils import run_bass_kernel_spmd

# Problem dims (hardcoded per contract)
B, T, E = 8, 1000, 512
U = 100
U1 = U + 1          # 101 joint positions
H, D, P = 2, 256, 256
J, V = 128, 88
BLANK = V - 1
N_CORES = 8

# Device tiling
SPAN = 20           # t-steps per combined lhsT tile (K = SPAN + U1 = 121)
CH = 5              # t-steps per pre-add matmul chunk (N = 505)
HALF = 10           # t-steps per PSUM tile (A and M both [*, 1024] = 2 banks)
STG = 40            # t-steps per staging tile / DMA store (1.42 MB)

F32 = mybir.dt.float32

_CACHE = {}


def _build_program():
    nc = bacc.Bacc("TRN2", target_bir_lowering=False, debug=False)

    encT = nc.dram_tensor("encT", [E, T], F32, kind="ExternalInput").ap()
    jw1enc = nc.dram_tensor("jw1enc", [E, J], F32, kind="ExternalInput").ap()
    jw2d = nc.dram_tensor("jw2d", [J, V], F32, kind="ExternalInput").ap()
    biasu = nc.dram_tensor("biasu", [U1, J], F32, kind="ExternalInput").ap()
    seld = nc.dram_tensor("seld", [SPAN + U1, SPAN * U1], F32, kind="ExternalInput").ap()
    out = nc.dram_tensor("out", [T, U1, V], F32, kind="ExternalOutput").ap()

    with tile.TileContext(nc) as tc:
        with (
            tc.tile_pool(name="singles", bufs=1) as singles,
            tc.tile_pool(name="hidp", bufs=3) as hidp,
            tc.tile_pool(name="stgp", bufs=2) as stgp,
            tc.tile_pool(name="psA", bufs=2, space="PSUM") as psA,
            tc.tile_pool(name="psM", bufs=2, space="PSUM") as psM,
        ):
            # ---- persistent SBUF tensors ----
            encT_sb = []
            for k in range(4):
                t_ = singles.tile([128, T], F32, tag=f"encT{k}")
                nc.sync.dma_start(out=t_[:, :], in_=encT[k * 128:(k + 1) * 128, :])
                encT_sb.append(t_)
            jw1_sb = []
            for k in range(4):
                t_ = singles.tile([128, J], F32, tag=f"jw1_{k}")
                nc.sync.dma_start(out=t_[:, :], in_=jw1enc[k * 128:(k + 1) * 128, :])
                jw1_sb.append(t_)
            jw2_sb = singles.tile([J, V], F32, tag="jw2")
            nc.sync.dma_start(out=jw2_sb[:, :], in_=jw2d[:, :])
            sel_sb = singles.tile([SPAN + U1, SPAN * U1], F32, tag="sel")
            nc.sync.dma_start(out=sel_sb[:, :], in_=seld[:, :])
            # two alternating combined lhsT tiles; bias rows are static
            combined = []
            for i in range(2):
                t_ = singles.tile([SPAN + U1, J], F32, tag=f"comb{i}")
                nc.sync.dma_start(out=t_[SPAN:SPAN + U1, :], in_=biasu[:, :])
                combined.append(t_)
            # enc_proj row-major, chunks of 100 t: row t = (t%100), col-block t//100
            enc_proj = singles.tile([100, 10 * J], F32, tag="encproj")

            # ---- prologue: enc_proj[t, j] = sum_e enc[t, e] * jw1enc[e, j] ----
            for ci in range(10):
                ep = psA.tile([100, J], F32, tag="A")
                for k in range(4):
                    nc.tensor.matmul(
                        ep[:, :],
                        encT_sb[k][:, ci * 100:(ci + 1) * 100],
                        jw1_sb[k][:, :],
                        start=(k == 0),
                        stop=(k == 3),
                    )
                nc.vector.tensor_copy(
                    out=enc_proj[:, ci * J:(ci + 1) * J], in_=ep[:, :]
                )

            # ---- main loop, software-skewed by one HALF ----
            n_half = T // HALF  # 100
            hid_tiles = [None] * n_half
            A_tiles = [None] * n_half
            M_tiles = [None] * n_half
            stg_tile = [None]

            def emit_front(h):
                # pre-add matmuls + tanh for half h
                t0 = h * HALF
                span_i = t0 // SPAN
                if t0 % SPAN == 0:
                    # load this span's 20 enc_proj rows into the combined tile
                    cb = combined[span_i % 2]
                    blk, p0 = t0 // 100, t0 % 100
                    nc.gpsimd.dma_start(
                        out=cb[0:SPAN, :],
                        in_=enc_proj[p0:p0 + SPAN, blk * J:(blk + 1) * J],
                    )
                cb = combined[span_i % 2]
                A = psA.tile([128, 1024], F32, tag="A")
                lh = (t0 % SPAN) // HALF  # 0 or 1: which pair of chunks in sel
                for c in range(2):
                    nc.tensor.matmul(
                        A[:, c * 512:c * 512 + CH * U1],
                        cb[:, :],
                        sel_sb[:, (lh * 2 + c) * CH * U1:(lh * 2 + c + 1) * CH * U1],
                        start=True,
                        stop=True,
                    )
                hid = hidp.tile([128, 2 * CH * U1], F32, tag="hid")
                nc.scalar.activation(
                    out=hid.rearrange("p (c x) -> p c x", c=2),
                    in_=A.rearrange("p (c x) -> p c x", c=2)[:, :, 0:CH * U1],
                    func=mybir.ActivationFunctionType.Tanh,
                )
                A_tiles[h] = A
                hid_tiles[h] = hid

            def emit_back(h):
                # joint matmuls + evacuation for half h
                hid = hid_tiles[h]
                M = psM.tile([U1, 1024], F32, tag="M")
                for m in range(HALF):
                    col = (m // CH) * 512 + (m % CH) * V
                    nc.tensor.matmul(
                        M[:, col:col + V],
                        hid[:, m * U1:(m + 1) * U1],
                        jw2_sb[:, :],
                        start=True,
                        stop=True,
                    )
                g_h = h % (STG // HALF)  # position within staging group
                if g_h == 0:
                    stg_tile[0] = stgp.tile([U1, STG * V], F32, tag="stg", name=f"stg{h}")
                stg = stg_tile[0]
                nc.vector.tensor_copy(
                    out=stg[:, h * HALF % STG * V:][:, 0:HALF * V].rearrange(
                        "p (b x) -> p b x", b=2
                    ),
                    in_=M.rearrange("p (b x) -> p b x", b=2)[:, :, 0:CH * V],
                )
                if g_h == STG // HALF - 1:
                    g = h // (STG // HALF)
                    nc.sync.dma_start(
                        out=out[g * STG:(g + 1) * STG, :, :].rearrange(
                            "t u v -> u t v"
                        ),
                        in_=stg.rearrange("p (t v) -> p t v", v=V),
                    )

            for h in range(n_half + 1):
                if h < n_half:
                    emit_front(h)
                if h >= 1:
                    emit_back(h - 1)

    nc.compile()
    return nc


def _host_pred_bias(targets_b, emb, pw1, pb1, pw2, pb2, jw1, jb1):
    """bias[u, j] = (pred @ jw1[E:] + jb1)[u, j] for the 101 joint positions."""
    ext = np.concatenate([np.full(H, BLANK, np.int64), targets_b.astype(np.int64)])
    e = np.concatenate([emb[ext[1:U1 + 1]], emb[ext[0:U1]]], axis=1)  # [101, 512]
    h = np.tanh(e @ pw1 + pb1)
    pred = np.tanh(h @ pw2 + pb2)
    return (pred @ jw1[E:] + jb1).astype(np.float32)  # [101, 128]


def _make_sel():
    sel = np.zeros((SPAN + U1, SPAN * U1), np.float32)
    for tl in range(SPAN):
        sel[tl, tl * U1:(tl + 1) * U1] = 1.0
        sel[SPAN:SPAN + U1, tl * U1:(tl + 1) * U1] += np.eye(U1, dtype=np.float32)
    return sel


def kernel(encoder_states, encoder_states_size, targets, targets_size,
           emb, pw1, pb1, pw2, pb2, jw1, jb1, jw2, jb2):
    if "nc" not in _CACHE:
        _CACHE["nc"] = _build_program()
    nc = _CACHE["nc"]

    encoder_states = np.asarray(encoder_states, dtype=np.float32)
    jw1 = np.asarray(jw1, dtype=np.float32)
    jw2_np = np.ascontiguousarray(np.asarray(jw2, dtype=np.float32))
    jw1enc = np.ascontiguousarray(jw1[:E])
    sel = _make_sel()

    in_maps = []
    for b in range(B):
        encT_b = np.ascontiguousarray(encoder_states[b].T)  # [E, T]
        bias_b = _host_pred_bias(
            np.asarray(targets[b]), np.asarray(emb, np.float32),
            np.asarray(pw1, np.float32), np.asarray(pb1, np.float32),
            np.asarray(pw2, np.float32), np.asarray(pb2, np.float32),
            jw1, np.asarray(jb1, np.float32),
        )
        in_maps.append({
            "encT": encT_b,
            "jw1enc": jw1enc,
            "jw2d": jw2_np,
            "biasu": bias_b,
            "seld": sel,
        })

    _CACHE["in_maps"] = in_maps
    res = run_bass_kernel_spmd(nc, in_maps, core_ids=list(range(N_CORES)))

    out = np.empty((B, T, U1, V), np.float32)
    for b in range(B):
        out[b] = res.results[b]["out"]
    out += np.asarray(jb2, np.float32)  # jb2 epilogue (host)
    # ragged masking (host epilogue)
    tsz = np.asarray(encoder_states_size).astype(np.int64)
    usz = np.asarray(targets_size).astype(np.int64) + 1
    for b in range(B):
        out[b, tsz[b]:, :, :] = 0.0
        out[b, :, usz[b]:, :] = 0.0
    return out


# revision 7
# speedup vs baseline: 41.9720x; 41.9720x over previous
"""FFNN-Transducer joint-lattice kernel for 8 Trainium2 NeuronCores.

Data-parallel over batch B=8 (one sample per core). The device computes the
dense T x (U+1) joint lattice:
    out[t,u,:] = tanh(enc_proj[t,:] + pred_bias[u,:]) @ jw2
where enc_proj = enc @ jw1[:E] is computed on-device and pred_bias[u,:]
(= pred @ jw1[E:] + jb1) comes from the tiny prediction network (done on host,
<0.3% of total FLOPs). jb2-add and the ragged masking are host epilogues.

TRN2 fp32 matmul runs at 1/4 rate (hi/lo 2-pass), so all TensorE-facing
tensors are fp16 (measured 4.2e-4 rel err vs 2.2e-3 for bf16); PSUM
accumulation, staging and the output stay fp32.

Device pipeline per core (t-major, 1000 t-steps):
  PE:  enc_projT prologue; per 5-t chunk one "selection" matmul that
       materializes A[j,(t,u)] = enc_proj[t,j] + bias[u,j] in PSUM
       (lhsT = [20 enc rows ; 101 bias rows], rhs = 0/1 selection matrix);
       per t one [128x128(pad)] x [128x88] joint matmul into PSUM (FWL).
  ACT: batched tanh PSUM->SBUF(fp16), 1010 elems/instr.
  DVE: PSUM->SBUF staging evacuation (the only legal PSUM->DRAM route).
  DMA: 1.42 MB contiguous stores (40 t-steps each).
"""

import os
import sys

for _p in ("/opt/trn_rl_repo", "/root/.axon_site/_ro/trn_rl_repo"):
    if os.path.isdir(_p) and _p not in sys.path:
        sys.path.append(_p)

import numpy as np

import concourse.bass as bass
import concourse.tile as tile
from concourse import bacc, mybir
from concourse.bass_utils import run_bass_kernel_spmd

# Problem dims (hardcoded per contract)
B, T, E = 8, 1000, 512
U = 100
U1 = U + 1          # 101 joint positions
H, D, P = 2, 256, 256
J, V = 128, 88
BLANK = V - 1
N_CORES = 8

# Device tiling
SPAN = 20           # t-steps per combined lhsT tile (K = SPAN + U1 = 121)
CH = 5              # t-steps per pre-add matmul chunk (N = 505)
HALF = 10           # t-steps per PSUM tile (A and M both [*, 1024] = 2 banks)
STG = 40            # t-steps per staging tile / DMA store (1.42 MB)
MPAD = 128          # joint-matmul lhsT padded to 128 cols => FWL fast load

F32 = mybir.dt.float32
F16 = mybir.dt.float16

_CACHE = {}


def _build_program(reps=1):
    nc = bacc.Bacc("TRN2", target_bir_lowering=False, debug=False)

    encT = nc.dram_tensor("encT", [E, T], F16, kind="ExternalInput").ap()
    jw1enc = nc.dram_tensor("jw1enc", [E, J], F16, kind="ExternalInput").ap()
    jw2d = nc.dram_tensor("jw2d", [J, V], F16, kind="ExternalInput").ap()
    biasu = nc.dram_tensor("biasu", [U1, J], F16, kind="ExternalInput").ap()
    seld = nc.dram_tensor("seld", [SPAN + U1, SPAN * U1], F16, kind="ExternalInput").ap()
    out = nc.dram_tensor("out", [T, U1, V], F32, kind="ExternalOutput").ap()

    with tile.TileContext(nc) as tc:
        with (
            tc.tile_pool(name="singles", bufs=1) as singles,
            tc.tile_pool(name="hidp", bufs=3) as hidp,
            tc.tile_pool(name="stgp", bufs=2) as stgp,
            tc.tile_pool(name="psA", bufs=2, space="PSUM") as psA,
            tc.tile_pool(name="psM", bufs=2, space="PSUM") as psM,
        ):
            # ---- persistent SBUF tensors ----
            encT_sb = []
            for k in range(4):
                t_ = singles.tile([128, T], F16, tag=f"encT{k}")
                nc.sync.dma_start(out=t_[:, :], in_=encT[k * 128:(k + 1) * 128, :])
                encT_sb.append(t_)
            jw1_sb = []
            for k in range(4):
                t_ = singles.tile([128, J], F16, tag=f"jw1_{k}")
                nc.sync.dma_start(out=t_[:, :], in_=jw1enc[k * 128:(k + 1) * 128, :])
                jw1_sb.append(t_)
            jw2_sb = singles.tile([J, V], F16, tag="jw2")
            nc.sync.dma_start(out=jw2_sb[:, :], in_=jw2d[:, :])
            sel_sb = singles.tile([SPAN + U1, SPAN * U1], F16, tag="sel")
            nc.sync.dma_start(out=sel_sb[:, :], in_=seld[:, :])
            # two alternating combined lhsT tiles; bias rows are static
            combined = []
            for i in range(2):
                t_ = singles.tile([SPAN + U1, J], F16, tag=f"comb{i}")
                nc.sync.dma_start(out=t_[SPAN:SPAN + U1, :], in_=biasu[:, :])
                combined.append(t_)
            # enc_proj row-major, chunks of 100 t: row t = (t%100), col-block t//100
            enc_proj = singles.tile([100, 10 * J], F16, tag="encproj")

            for rep in range(reps):
                _emit_rep(nc, tc, singles, hidp, stgp, psA, psM,
                          encT_sb, jw1_sb, jw2_sb, sel_sb, combined, enc_proj,
                          out, rep)

    nc.compile()
    return nc


def _emit_rep(nc, tc, singles, hidp, stgp, psA, psM,
              encT_sb, jw1_sb, jw2_sb, sel_sb, combined, enc_proj, out, rep):
    # ---- prologue: enc_proj[t, j] = sum_e enc[t, e] * jw1enc[e, j] ----
    for ci in range(10):
        ep = psA.tile([100, J], F32, tag="A", name=f"ep{rep}_{ci}")
        for k in range(4):
            nc.tensor.matmul(
                ep[:, :],
                encT_sb[k][:, ci * 100:(ci + 1) * 100],
                jw1_sb[k][:, :],
                start=(k == 0),
                stop=(k == 3),
            )
        nc.vector.tensor_copy(out=enc_proj[:, ci * J:(ci + 1) * J], in_=ep[:, :])

    # ---- main loop, software-skewed by one HALF ----
    n_half = T // HALF  # 100
    hid_tiles = [None] * n_half
    stg_tile = [None]

    def emit_front(h):
        # pre-add matmuls + tanh for half h
        t0 = h * HALF
        span_i = t0 // SPAN
        if t0 % SPAN == 0:
            # load this span's 20 enc_proj rows into the combined tile
            cb = combined[span_i % 2]
            blk, p0 = t0 // 100, t0 % 100
            nc.sync.dma_start(
                out=cb[0:SPAN, :],
                in_=enc_proj[p0:p0 + SPAN, blk * J:(blk + 1) * J],
            )
        cb = combined[span_i % 2]
        A = psA.tile([128, 1024], F32, tag="A", name=f"A{rep}_{h}")
        lh = (t0 % SPAN) // HALF  # 0 or 1: which pair of chunks in sel
        for c in range(2):
            nc.tensor.matmul(
                A[:, c * 512:c * 512 + CH * U1],
                cb[:, :],
                sel_sb[:, (lh * 2 + c) * CH * U1:(lh * 2 + c + 1) * CH * U1],
                start=True,
                stop=True,
            )
        # fp16 hid, padded so the joint matmul can take 128-col slices (FWL)
        hid = hidp.tile([128, HALF * U1 + (MPAD - U1)], F16, tag="hid",
                        name=f"hid{rep}_{h}")
        nc.gpsimd.memset(hid[:, HALF * U1:], 0.0)
        nc.scalar.activation(
            out=hid[:, 0:HALF * U1].rearrange("p (c x) -> p c x", c=2),
            in_=A.rearrange("p (c x) -> p c x", c=2)[:, :, 0:CH * U1],
            func=mybir.ActivationFunctionType.Tanh,
        )
        hid_tiles[h] = hid

    def emit_back(h):
        # joint matmuls + evacuation for half h
        hid = hid_tiles[h]
        M = psM.tile([128, 1024], F32, tag="M", name=f"M{rep}_{h}")
        for m in range(HALF):
            col = (m // CH) * 512 + (m % CH) * V
            nc.tensor.matmul(
                M[:, col:col + V],
                hid[:, m * U1:m * U1 + MPAD],
                jw2_sb[:, :],
                start=True,
                stop=True,
            )
        g_h = h % (STG // HALF)  # position within staging group
        if g_h == 0:
            stg_tile[0] = stgp.tile([U1, STG * V], F32, tag="stg",
                                    name=f"stg{rep}_{h}")
        stg = stg_tile[0]
        nc.vector.tensor_copy(
            out=stg[:, g_h * HALF * V:(g_h + 1) * HALF * V].rearrange(
                "p (b x) -> p b x", b=2
            ),
            in_=M[0:U1, :].rearrange("p (b x) -> p b x", b=2)[:, :, 0:CH * V],
        )
        if g_h == STG // HALF - 1:
            g = h // (STG // HALF)
            nc.sync.dma_start(
                out=out[g * STG:(g + 1) * STG, :, :].rearrange("t u v -> u t v"),
                in_=stg.rearrange("p (t v) -> p t v", v=V),
            )

    for h in range(n_half + 1):
        if h < n_half:
            emit_front(h)
        if h >= 1:
            emit_back(h - 1)


def _host_pred_bias(targets_b, emb, pw1, pb1, pw2, pb2, jw1, jb1):
    """bias[u, j] = (pred @ jw1[E:] + jb1)[u, j] for the 101 joint positions."""
    ext = np.concatenate([np.full(H, BLANK, np.int64), targets_b.astype(np.int64)])
    e = np.concatenate([emb[ext[1:U1 + 1]], emb[ext[0:U1]]], axis=1)  # [101, 512]
    h = np.tanh(e @ pw1 + pb1)
    pred = np.tanh(h @ pw2 + pb2)
    return (pred @ jw1[E:] + jb1).astype(np.float32)  # [101, 128]


def _make_sel():
    sel = np.zeros((SPAN + U1, SPAN * U1), np.float16)
    for tl in range(SPAN):
        sel[tl, tl * U1:(tl + 1) * U1] = 1.0
        sel[SPAN:SPAN + U1, tl * U1:(tl + 1) * U1] += np.eye(U1, dtype=np.float16)
    return sel


def _make_in_maps(encoder_states, targets, emb, pw1, pb1, pw2, pb2, jw1, jb1, jw2):
    encoder_states = np.asarray(encoder_states, dtype=np.float32)
    jw1 = np.asarray(jw1, dtype=np.float32)
    jw2_np = np.ascontiguousarray(np.asarray(jw2, dtype=np.float32)).astype(np.float16)
    jw1enc = np.ascontiguousarray(jw1[:E]).astype(np.float16)
    sel = _make_sel()

    in_maps = []
    for b in range(B):
        encT_b = np.ascontiguousarray(encoder_states[b].T).astype(np.float16)
        bias_b = _host_pred_bias(
            np.asarray(targets[b]), np.asarray(emb, np.float32),
            np.asarray(pw1, np.float32), np.asarray(pb1, np.float32),
            np.asarray(pw2, np.float32), np.asarray(pb2, np.float32),
            jw1, np.asarray(jb1, np.float32),
        ).astype(np.float16)
        in_maps.append({
            "encT": encT_b,
            "jw1enc": jw1enc,
            "jw2d": jw2_np,
            "biasu": bias_b,
            "seld": sel,
        })
    return in_maps


def kernel(encoder_states, encoder_states_size, targets, targets_size,
           emb, pw1, pb1, pw2, pb2, jw1, jb1, jw2, jb2):
    if "nc" not in _CACHE:
        _CACHE["nc"] = _build_program()
    nc = _CACHE["nc"]

    in_maps = _make_in_maps(encoder_states, targets, emb, pw1, pb1, pw2, pb2,
                            jw1, jb1, jw2)
    _CACHE["in_maps"] = in_maps
    res = run_bass_kernel_spmd(nc, in_maps, core_ids=list(range(N_CORES)))

    out = np.empty((B, T, U1, V), np.float32)
    for b in range(B):
        out[b] = res.results[b]["out"]
    out += np.asarray(jb2, np.float32)  # jb2 epilogue (host)
    # ragged masking (host epilogue)
    tsz = np.asarray(encoder_states_size).astype(np.int64)
    usz = np.asarray(targets_size).astype(np.int64) + 1
    for b in range(B):
        out[b, tsz[b]:, :, :] = 0.0
        out[b, :, usz[b]:, :] = 0.0
    return out


# revision 8
# speedup vs baseline: 164.0705x; 3.9091x over previous
"""FFNN-Transducer joint-lattice kernel for 8 Trainium2 NeuronCores.

Data-parallel over batch B=8 (one sample per core). The device computes the
dense T x (U+1) joint lattice:
    out[t,u,:] = tanh(enc_proj[t,:] + pred_bias[u,:]) @ jw2
where enc_proj = enc @ jw1[:E] is computed on-device and pred_bias[u,:]
(= pred @ jw1[E:] + jb1) comes from the tiny prediction network (done on host,
<0.3% of total FLOPs). jb2-add and the ragged masking are host epilogues.

TRN2 fp32 matmul runs at 1/4 rate (hi/lo 2-pass), so all TensorE-facing
tensors are fp16 (measured 4.2e-4 rel err vs 2.2e-3 for bf16); PSUM
accumulation, staging and the output stay fp32.

Device pipeline per core, in t-blocks of 128 (T padded to 1024):
  PE:   enc_proj prologue (4-acc matmuls per 128-t chunk);
        per 4-t chunk one "selection" matmul materializing
        A[j,(t,u)] = enc_proj[t,j] + bias[u,j] in PSUM
        (lhsT = [16 enc rows ; 101 bias rows], rhs = 0/1 selection);
        per u one [128x128] x [128x88] joint matmul into PSUM (FWL).
  ACT:  batched tanh PSUM->SBUF fp16 that simultaneously TRANSPOSES the
        lattice from t-major (matmul chunk order) to u-major via a strided
        output AP - this makes the joint matmul's lhsT slices contiguous.
  DVE:  PSUM->SBUF staging evacuation into [t-partition, (u,v)] layout.
  DMA:  per-block stores [128, 8888] where every partition's 35.5KB is
        contiguous in DRAM (large descriptors -> full HBM bandwidth).
"""

import os
import sys

for _p in ("/opt/trn_rl_repo", "/root/.axon_site/_ro/trn_rl_repo"):
    if os.path.isdir(_p) and _p not in sys.path:
        sys.path.append(_p)

import numpy as np

import concourse.bass as bass
import concourse.tile as tile
from concourse import bacc, mybir
from concourse.bass_utils import run_bass_kernel_spmd

# Problem dims (hardcoded per contract)
B, T, E = 8, 1000, 512
U = 100
U1 = U + 1          # 101 joint positions
H, D, P = 2, 256, 256
J, V = 128, 88
BLANK = V - 1
N_CORES = 8

# Device tiling
TP = 1024           # padded T (8 blocks of 128)
TB = 128            # t-steps per block (= joint-matmul lhsT cols, FWL)
NB = TP // TB       # 8 blocks
HALF = 8            # t-steps per A-PSUM tile ([128, 1024] = 2 banks)
CH = 4              # t-steps per pre-add matmul chunk (N = 404)
SPAN = 16           # t-steps per combined lhsT tile (K = SPAN + U1 = 117)
UG = 10             # u-steps per M-PSUM tile ([128, 1024] = 2 banks)
NUG = 11            # u-groups per block (10 full + 1 leftover)

F32 = mybir.dt.float32
F16 = mybir.dt.float16

_CACHE = {}


def _build_program(reps=1):
    nc = bacc.Bacc("TRN2", target_bir_lowering=False, debug=False)

    encT = nc.dram_tensor("encT", [E, TP], F16, kind="ExternalInput").ap()
    jw1enc = nc.dram_tensor("jw1enc", [E, J], F16, kind="ExternalInput").ap()
    jw2d = nc.dram_tensor("jw2d", [J, V], F16, kind="ExternalInput").ap()
    biasu = nc.dram_tensor("biasu", [U1, J], F16, kind="ExternalInput").ap()
    seld = nc.dram_tensor("seld", [SPAN + U1, SPAN * U1], F16, kind="ExternalInput").ap()
    out = nc.dram_tensor("out", [T, U1 * V], F32, kind="ExternalOutput").ap()

    with tile.TileContext(nc) as tc:
        with (
            tc.tile_pool(name="singles", bufs=1) as singles,
            tc.tile_pool(name="hidp", bufs=2) as hidp,
            tc.tile_pool(name="stgp", bufs=2) as stgp,
            tc.tile_pool(name="psA", bufs=2, space="PSUM") as psA,
            tc.tile_pool(name="psM", bufs=2, space="PSUM") as psM,
        ):
            # ---- persistent SBUF tensors ----
            encT_sb = []
            for k in range(4):
                t_ = singles.tile([128, TP], F16, tag=f"encT{k}")
                nc.sync.dma_start(out=t_[:, :], in_=encT[k * 128:(k + 1) * 128, :])
                encT_sb.append(t_)
            jw1_sb = []
            for k in range(4):
                t_ = singles.tile([128, J], F16, tag=f"jw1_{k}")
                nc.sync.dma_start(out=t_[:, :], in_=jw1enc[k * 128:(k + 1) * 128, :])
                jw1_sb.append(t_)
            jw2_sb = singles.tile([J, V], F16, tag="jw2")
            nc.sync.dma_start(out=jw2_sb[:, :], in_=jw2d[:, :])
            sel_sb = singles.tile([SPAN + U1, SPAN * U1], F16, tag="sel")
            nc.sync.dma_start(out=sel_sb[:, :], in_=seld[:, :])
            # two alternating combined lhsT tiles; bias rows are static
            combined = []
            for i in range(2):
                t_ = singles.tile([SPAN + U1, J], F16, tag=f"comb{i}")
                nc.sync.dma_start(out=t_[SPAN:SPAN + U1, :], in_=biasu[:, :])
                combined.append(t_)
            # enc_proj row-major: [t % 128, (t//128)*J + j] (block-aligned)
            enc_proj = singles.tile([128, NB * J], F16, tag="encproj")

            for rep in range(reps):
                _emit_rep(nc, hidp, stgp, psA, psM,
                          encT_sb, jw1_sb, jw2_sb, sel_sb, combined, enc_proj,
                          out, rep)

    nc.compile()
    return nc


def _emit_rep(nc, hidp, stgp, psA, psM,
              encT_sb, jw1_sb, jw2_sb, sel_sb, combined, enc_proj, out, rep):
    # ---- prologue: enc_proj[t, j] = sum_e enc[t, e] * jw1enc[e, j] ----
    for cb in range(NB):
        ep = psA.tile([TB, J], F32, tag="A", name=f"ep{rep}_{cb}")
        for k in range(4):
            nc.tensor.matmul(
                ep[:, :],
                encT_sb[k][:, cb * TB:(cb + 1) * TB],
                jw1_sb[k][:, :],
                start=(k == 0),
                stop=(k == 3),
            )
        nc.vector.tensor_copy(out=enc_proj[:, cb * J:(cb + 1) * J], in_=ep[:, :])

    hid_tiles = [None] * NB
    stg_tiles = [None] * NB

    def front(b, step):
        # pre-add matmuls + tanh(+transpose) for half (b, step)
        t0 = b * TB + step * HALF
        if step == 0:
            hid_tiles[b] = hidp.tile([128, U1 * TB], F16, tag="hid",
                                     name=f"hid{rep}_{b}")
        hid2 = hid_tiles[b]
        span_i = t0 // SPAN
        cb_t = combined[span_i % 2]
        if t0 % SPAN == 0:
            # load this span's 16 enc_proj rows into the combined tile
            blk, p0 = t0 // TB, t0 % TB
            nc.sync.dma_start(
                out=cb_t[0:SPAN, :],
                in_=enc_proj[p0:p0 + SPAN, blk * J:(blk + 1) * J],
            )
        A = psA.tile([128, 1024], F32, tag="A", name=f"A{rep}_{b}_{step}")
        lh = (t0 % SPAN) // HALF  # 0 or 1: which pair of chunks in sel
        for c in range(2):
            nc.tensor.matmul(
                A[:, c * 512:c * 512 + CH * U1],
                cb_t[:, :],
                sel_sb[:, (lh * 2 + c) * CH * U1:(lh * 2 + c + 1) * CH * U1],
                start=True,
                stop=True,
            )
        # tanh + lattice transpose: in (c, tl, u) t-major -> out u-major
        base = step * HALF
        in_ap = A.rearrange("p (c x) -> p c x", c=2)[:, :, 0:CH * U1].rearrange(
            "p c (tl u) -> p c tl u", u=U1)
        out_ap = hid2.rearrange("p (u t) -> p u t", t=TB)[
            :, :, base:base + HALF].rearrange("p u (c tl) -> p c tl u", c=2)
        nc.scalar.activation(
            out=out_ap, in_=in_ap,
            func=mybir.ActivationFunctionType.Tanh,
        )

    def back(b, ug):
        # joint matmuls + evacuation for u-group ug of block b
        hid2 = hid_tiles[b]
        if ug == 0:
            stg_tiles[b] = stgp.tile([TB, U1 * V], F32, tag="stg",
                                     name=f"stg{rep}_{b}")
        stg = stg_tiles[b]
        u0 = ug * UG
        n_u = UG if ug < NUG - 1 else U1 - u0
        M = psM.tile([TB, 1024], F32, tag="M", name=f"M{rep}_{b}_{ug}")
        for i in range(n_u):
            col = (i // 5) * 512 + (i % 5) * V
            nc.tensor.matmul(
                M[:, col:col + V],
                hid2[:, (u0 + i) * TB:(u0 + i + 1) * TB],
                jw2_sb[:, :],
                start=True,
                stop=True,
            )
        if n_u == UG:
            nc.vector.tensor_copy(
                out=stg[:, u0 * V:(u0 + UG) * V].rearrange("p (bk x) -> p bk x", bk=2),
                in_=M.rearrange("p (bk x) -> p bk x", bk=2)[:, :, 0:5 * V],
            )
        else:
            nc.vector.tensor_copy(
                out=stg[:, u0 * V:(u0 + n_u) * V],
                in_=M[:, 0:n_u * V],
            )
        if ug == NUG - 1:
            n_t = min(TB, T - b * TB)
            nc.sync.dma_start(
                out=out[b * TB:b * TB + n_t, :],
                in_=stg[0:n_t, :],
            )

    # software-pipelined emission: block b's fronts interleave with b-1's backs
    for b in range(NB):
        for step in range(TB // HALF):  # 16
            front(b, step)
            if b >= 1 and step < NUG:
                back(b - 1, step)
    for ug in range(NUG):
        back(NB - 1, ug)


def _host_pred_bias(targets_b, emb, pw1, pb1, pw2, pb2, jw1, jb1):
    """bias[u, j] = (pred @ jw1[E:] + jb1)[u, j] for the 101 joint positions."""
    ext = np.concatenate([np.full(H, BLANK, np.int64), targets_b.astype(np.int64)])
    e = np.concatenate([emb[ext[1:U1 + 1]], emb[ext[0:U1]]], axis=1)  # [101, 512]
    h = np.tanh(e @ pw1 + pb1)
    pred = np.tanh(h @ pw2 + pb2)
    return (pred @ jw1[E:] + jb1).astype(np.float32)  # [101, 128]


def _make_sel():
    sel = np.zeros((SPAN + U1, SPAN * U1), np.float16)
    for tl in range(SPAN):
        sel[tl, tl * U1:(tl + 1) * U1] = 1.0
        sel[SPAN:SPAN + U1, tl * U1:(tl + 1) * U1] += np.eye(U1, dtype=np.float16)
    return sel


def _make_in_maps(encoder_states, targets, emb, pw1, pb1, pw2, pb2, jw1, jb1, jw2):
    encoder_states = np.asarray(encoder_states, dtype=np.float32)
    jw1 = np.asarray(jw1, dtype=np.float32)
    jw2_np = np.ascontiguousarray(np.asarray(jw2, dtype=np.float32)).astype(np.float16)
    jw1enc = np.ascontiguousarray(jw1[:E]).astype(np.float16)
    sel = _make_sel()

    in_maps = []
    for b in range(B):
        encT_b = np.zeros((E, TP), np.float16)
        encT_b[:, :T] = encoder_states[b].T.astype(np.float16)
        bias_b = _host_pred_bias(
            np.asarray(targets[b]), np.asarray(emb, np.float32),
            np.asarray(pw1, np.float32), np.asarray(pb1, np.float32),
            np.asarray(pw2, np.float32), np.asarray(pb2, np.float32),
            jw1, np.asarray(jb1, np.float32),
        ).astype(np.float16)
        in_maps.append({
            "encT": encT_b,
            "jw1enc": jw1enc,
            "jw2d": jw2_np,
            "biasu": bias_b,
            "seld": sel,
        })
    return in_maps


def kernel(encoder_states, encoder_states_size, targets, targets_size,
           emb, pw1, pb1, pw2, pb2, jw1, jb1, jw2, jb2):
    if "nc" not in _CACHE:
        _CACHE["nc"] = _build_program()
    nc = _CACHE["nc"]

    in_maps = _make_in_maps(encoder_states, targets, emb, pw1, pb1, pw2, pb2,
                            jw1, jb1, jw2)
    _CACHE["in_maps"] = in_maps
    res = run_bass_kernel_spmd(nc, in_maps, core_ids=list(range(N_CORES)))

    out = np.empty((B, T, U1, V), np.float32)
    for b in range(B):
        out[b] = res.results[b]["out"].reshape(T, U1, V)
    out += np.asarray(jb2, np.float32)  # jb2 epilogue (host)
    # ragged masking (host epilogue)
    tsz = np.asarray(encoder_states_size).astype(np.int64)
    usz = np.asarray(targets_size).astype(np.int64) + 1
    for b in range(B):
        out[b, tsz[b]:, :, :] = 0.0
        out[b, :, usz[b]:, :] = 0.0
    return out


# revision 10
# speedup vs baseline: 254.7607x; 1.5528x over previous
"""FFNN-Transducer joint-lattice kernel for 8 Trainium2 NeuronCores.

Data-parallel over batch B=8 (one sample per core). The device computes the
dense T x (U+1) joint lattice:
    out[t,u,:] = tanh(enc_proj[t,:] + pred_bias[u,:]) @ jw2
where enc_proj = enc @ jw1[:E] is computed on-device and pred_bias[u,:]
(= pred @ jw1[E:] + jb1) comes from the tiny prediction network (done on host,
<0.3% of total FLOPs). jb2-add and the ragged masking are host epilogues.

TRN2 fp32 matmul runs at 1/4 rate (hi/lo 2-pass), so all TensorE-facing
tensors are fp16 (measured 4.2e-4 rel err vs 2.2e-3 for bf16); PSUM
accumulation, staging and the output stay fp32.

Device pipeline per core, in t-blocks of 128 (T padded to 1024):
  PE:   enc_proj prologue (4-acc matmuls per 128-t chunk);
        per 4-t chunk one "selection" matmul materializing
        A[j,(t,u)] = enc_proj[t,j] + bias[u,j] in PSUM
        (lhsT = [16 enc rows ; 101 bias rows], rhs = 0/1 selection);
        per u one [128x128] x [128x88] joint matmul into PSUM (FWL).
  ACT:  batched tanh PSUM->SBUF fp16 that simultaneously TRANSPOSES the
        lattice from t-major (matmul chunk order) to u-major via a strided
        output AP - this makes the joint matmul's lhsT slices contiguous.
  DVE:  PSUM->SBUF staging evacuation into [t-partition, (u,v)] layout.
  DMA:  per-block stores [128, 8888] where every partition's 35.5KB is
        contiguous in DRAM (large descriptors -> full HBM bandwidth).
"""

import os
import sys

for _p in ("/opt/trn_rl_repo", "/root/.axon_site/_ro/trn_rl_repo"):
    if os.path.isdir(_p) and _p not in sys.path:
        sys.path.append(_p)

import numpy as np

import concourse.bass as bass
import concourse.tile as tile
from concourse import bacc, mybir
from concourse.bass_utils import run_bass_kernel_spmd

# Problem dims (hardcoded per contract)
B, T, E = 8, 1000, 512
U = 100
U1 = U + 1          # 101 joint positions
H, D, P = 2, 256, 256
J, V = 128, 88
BLANK = V - 1
N_CORES = 8

# Device tiling
TP = 1024           # padded T (8 blocks of 128)
TB = 128            # t-steps per block (= joint-matmul lhsT cols, FWL)
NB = TP // TB       # 8 blocks
HALF = 8            # t-steps per A-PSUM tile ([128, 1024] = 2 banks)
CH = 4              # t-steps per pre-add matmul chunk (N = 404)
SPAN = 16           # t-steps per combined lhsT tile (K = SPAN + U1 = 117)
UG = 10             # u-steps per M-PSUM tile ([128, 1024] = 2 banks)
NUG = 11            # u-groups per block (10 full + 1 leftover)

F32 = mybir.dt.float32
F16 = mybir.dt.float16

_CACHE = {}


def _build_program(reps=1):
    nc = bacc.Bacc("TRN2", target_bir_lowering=False, debug=False)

    encT = nc.dram_tensor("encT", [E, TP], F16, kind="ExternalInput").ap()
    jw1enc = nc.dram_tensor("jw1enc", [E, J], F16, kind="ExternalInput").ap()
    jw2d = nc.dram_tensor("jw2d", [J, V], F16, kind="ExternalInput").ap()
    biasu = nc.dram_tensor("biasu", [U1, J], F16, kind="ExternalInput").ap()
    seld = nc.dram_tensor("seld", [SPAN + U1, SPAN * U1], F16, kind="ExternalInput").ap()
    out = nc.dram_tensor("out", [T, U1 * V], F32, kind="ExternalOutput").ap()

    with tile.TileContext(nc) as tc:
        with (
            tc.tile_pool(name="singles", bufs=1) as singles,
            tc.tile_pool(name="hidp", bufs=2) as hidp,
            tc.tile_pool(name="stgp", bufs=2) as stgp,
            tc.tile_pool(name="psA", bufs=2, space="PSUM") as psA,
            tc.tile_pool(name="psM", bufs=2, space="PSUM") as psM,
        ):
            # ---- persistent SBUF tensors ----
            encT_sb = []
            for k in range(4):
                t_ = singles.tile([128, TP], F16, tag=f"encT{k}")
                nc.sync.dma_start(out=t_[:, :], in_=encT[k * 128:(k + 1) * 128, :])
                encT_sb.append(t_)
            jw1_sb = []
            for k in range(4):
                t_ = singles.tile([128, J], F16, tag=f"jw1_{k}")
                nc.sync.dma_start(out=t_[:, :], in_=jw1enc[k * 128:(k + 1) * 128, :])
                jw1_sb.append(t_)
            jw2_sb = singles.tile([J, V], F16, tag="jw2")
            nc.sync.dma_start(out=jw2_sb[:, :], in_=jw2d[:, :])
            sel_sb = singles.tile([SPAN + U1, SPAN * U1], F16, tag="sel")
            nc.sync.dma_start(out=sel_sb[:, :], in_=seld[:, :])
            # two alternating combined lhsT tiles; bias rows are static
            combined = []
            for i in range(2):
                t_ = singles.tile([SPAN + U1, J], F16, tag=f"comb{i}")
                nc.sync.dma_start(out=t_[SPAN:SPAN + U1, :], in_=biasu[:, :])
                combined.append(t_)
            # enc_proj row-major: [t % 128, (t//128)*J + j] (block-aligned)
            enc_proj = singles.tile([128, NB * J], F16, tag="encproj")

            for rep in range(reps):
                _emit_rep(nc, hidp, stgp, psA, psM,
                          encT_sb, jw1_sb, jw2_sb, sel_sb, combined, enc_proj,
                          out, rep)

    nc.compile()
    return nc


def _emit_rep(nc, hidp, stgp, psA, psM,
              encT_sb, jw1_sb, jw2_sb, sel_sb, combined, enc_proj, out, rep):
    # ---- prologue: enc_proj[t, j] = sum_e enc[t, e] * jw1enc[e, j] ----
    for cb in range(NB):
        ep = psA.tile([TB, J], F32, tag="A", name=f"ep{rep}_{cb}")
        for k in range(4):
            nc.tensor.matmul(
                ep[:, :],
                encT_sb[k][:, cb * TB:(cb + 1) * TB],
                jw1_sb[k][:, :],
                start=(k == 0),
                stop=(k == 3),
            )
        nc.vector.tensor_copy(out=enc_proj[:, cb * J:(cb + 1) * J], in_=ep[:, :])

    hid_tiles = [None] * NB
    stg_tiles = [None] * NB

    def front(b, step):
        # pre-add matmuls + tanh(+transpose) for half (b, step)
        t0 = b * TB + step * HALF
        if step == 0:
            hid_tiles[b] = hidp.tile([128, U1 * TB], F16, tag="hid",
                                     name=f"hid{rep}_{b}")
        hid2 = hid_tiles[b]
        span_i = t0 // SPAN
        cb_t = combined[span_i % 2]
        if t0 % SPAN == 0:
            # load this span's 16 enc_proj rows into the combined tile
            blk, p0 = t0 // TB, t0 % TB
            nc.sync.dma_start(
                out=cb_t[0:SPAN, :],
                in_=enc_proj[p0:p0 + SPAN, blk * J:(blk + 1) * J],
            )
        A = psA.tile([128, 1024], F32, tag="A", name=f"A{rep}_{b}_{step}")
        lh = (t0 % SPAN) // HALF  # 0 or 1: which pair of chunks in sel
        for c in range(2):
            nc.tensor.matmul(
                A[:, c * 512:c * 512 + CH * U1],
                cb_t[:, :],
                sel_sb[:, (lh * 2 + c) * CH * U1:(lh * 2 + c + 1) * CH * U1],
                start=True,
                stop=True,
            )
        # tanh, contiguous t-major write; the joint matmul reads strided
        base = step * HALF
        nc.scalar.activation(
            out=hid2[:, base * U1:(base + HALF) * U1].rearrange(
                "p (c x) -> p c x", c=2),
            in_=A.rearrange("p (c x) -> p c x", c=2)[:, :, 0:CH * U1],
            func=mybir.ActivationFunctionType.Tanh,
        )

    def back(b, ug):
        # joint matmuls + evacuation for u-group ug of block b
        hid2 = hid_tiles[b]
        if ug == 0:
            stg_tiles[b] = stgp.tile([TB, U1 * V], F32, tag="stg",
                                     name=f"stg{rep}_{b}")
        stg = stg_tiles[b]
        u0 = ug * UG
        n_u = UG if ug < NUG - 1 else U1 - u0
        M = psM.tile([TB, 1024], F32, tag="M", name=f"M{rep}_{b}_{ug}")
        hid_ut = hid2.rearrange("p (t u) -> p u t", u=U1)  # [128, 101, 128] strided
        for i in range(n_u):
            col = (i // 5) * 512 + (i % 5) * V
            nc.tensor.matmul(
                M[:, col:col + V],
                hid_ut[:, u0 + i, :],
                jw2_sb[:, :],
                start=True,
                stop=True,
            )
        if n_u == UG:
            nc.vector.tensor_copy(
                out=stg[:, u0 * V:(u0 + UG) * V].rearrange("p (bk x) -> p bk x", bk=2),
                in_=M.rearrange("p (bk x) -> p bk x", bk=2)[:, :, 0:5 * V],
            )
        else:
            nc.vector.tensor_copy(
                out=stg[:, u0 * V:(u0 + n_u) * V],
                in_=M[:, 0:n_u * V],
            )
        if ug == NUG - 1:
            n_t = min(TB, T - b * TB)
            nc.sync.dma_start(
                out=out[b * TB:b * TB + n_t, :],
                in_=stg[0:n_t, :],
            )

    # software-pipelined emission: block b's fronts interleave with b-1's backs
    for b in range(NB):
        for step in range(TB // HALF):  # 16
            front(b, step)
            if b >= 1 and step < NUG:
                back(b - 1, step)
    for ug in range(NUG):
        back(NB - 1, ug)


def _host_pred_bias(targets_b, emb, pw1, pb1, pw2, pb2, jw1, jb1):
    """bias[u, j] = (pred @ jw1[E:] + jb1)[u, j] for the 101 joint positions."""
    ext = np.concatenate([np.full(H, BLANK, np.int64), targets_b.astype(np.int64)])
    e = np.concatenate([emb[ext[1:U1 + 1]], emb[ext[0:U1]]], axis=1)  # [101, 512]
    h = np.tanh(e @ pw1 + pb1)
    pred = np.tanh(h @ pw2 + pb2)
    return (pred @ jw1[E:] + jb1).astype(np.float32)  # [101, 128]


def _make_sel():
    sel = np.zeros((SPAN + U1, SPAN * U1), np.float16)
    for tl in range(SPAN):
        sel[tl, tl * U1:(tl + 1) * U1] = 1.0
        sel[SPAN:SPAN + U1, tl * U1:(tl + 1) * U1] += np.eye(U1, dtype=np.float16)
    return sel


def _make_in_maps(encoder_states, targets, emb, pw1, pb1, pw2, pb2, jw1, jb1, jw2):
    encoder_states = np.asarray(encoder_states, dtype=np.float32)
    jw1 = np.asarray(jw1, dtype=np.float32)
    jw2_np = np.ascontiguousarray(np.asarray(jw2, dtype=np.float32)).astype(np.float16)
    jw1enc = np.ascontiguousarray(jw1[:E]).astype(np.float16)
    sel = _make_sel()

    in_maps = []
    for b in range(B):
        encT_b = np.zeros((E, TP), np.float16)
        encT_b[:, :T] = encoder_states[b].T.astype(np.float16)
        bias_b = _host_pred_bias(
            np.asarray(targets[b]), np.asarray(emb, np.float32),
            np.asarray(pw1, np.float32), np.asarray(pb1, np.float32),
            np.asarray(pw2, np.float32), np.asarray(pb2, np.float32),
            jw1, np.asarray(jb1, np.float32),
        ).astype(np.float16)
        in_maps.append({
            "encT": encT_b,
            "jw1enc": jw1enc,
            "jw2d": jw2_np,
            "biasu": bias_b,
            "seld": sel,
        })
    return in_maps


def kernel(encoder_states, encoder_states_size, targets, targets_size,
           emb, pw1, pb1, pw2, pb2, jw1, jb1, jw2, jb2):
    if "nc" not in _CACHE:
        _CACHE["nc"] = _build_program()
    nc = _CACHE["nc"]

    in_maps = _make_in_maps(encoder_states, targets, emb, pw1, pb1, pw2, pb2,
                            jw1, jb1, jw2)
    _CACHE["in_maps"] = in_maps
    res = run_bass_kernel_spmd(nc, in_maps, core_ids=list(range(N_CORES)))

    out = np.empty((B, T, U1, V), np.float32)
    for b in range(B):
        out[b] = res.results[b]["out"].reshape(T, U1, V)
    out += np.asarray(jb2, np.float32)  # jb2 epilogue (host)
    # ragged masking (host epilogue)
    tsz = np.asarray(encoder_states_size).astype(np.int64)
    usz = np.asarray(targets_size).astype(np.int64) + 1
    for b in range(B):
        out[b, tsz[b]:, :, :] = 0.0
        out[b, :, usz[b]:, :] = 0.0
    return out


# revision 12
# speedup vs baseline: 1613.9182x; 6.3350x over previous
"""FFNN-Transducer joint-lattice kernel for 8 Trainium2 NeuronCores.

Data-parallel over batch B=8 (one sample per core). The device computes the
dense T x (U+1) joint lattice:
    out[t,u,:] = tanh(enc_proj[t,:] + pred_bias[u,:]) @ jw2
where enc_proj = enc @ jw1[:E] is computed on-device and pred_bias[u,:]
(= pred @ jw1[E:] + jb1) comes from the tiny prediction network (done on host,
<0.3% of total FLOPs). jb2-add and the ragged masking are host epilogues.

TRN2 fp32 matmul runs at 1/4 rate (hi/lo 2-pass), so all TensorE-facing
tensors are fp16 (measured 4.2e-4 rel err vs 2.2e-3 for bf16); PSUM
accumulation, staging and the output stay fp32.

Device pipeline per core, in t-blocks of 128 (T padded to 1024):
  PE:   enc_proj prologue (4-acc matmuls per 128-t chunk);
        per 4-t chunk one "selection" matmul materializing
        A[j,(t,u)] = enc_proj[t,j] + bias[u,j] in PSUM
        (lhsT = [16 enc rows ; 101 bias rows], rhs = 0/1 selection);
        per u one [128x128] x [128x88] joint matmul into PSUM (FWL).
  ACT:  batched tanh PSUM->SBUF fp16 that simultaneously TRANSPOSES the
        lattice from t-major (matmul chunk order) to u-major via a strided
        output AP - this makes the joint matmul's lhsT slices contiguous.
  DVE:  PSUM->SBUF staging evacuation into [t-partition, (u,v)] layout.
  DMA:  per-block stores [128, 8888] where every partition's 35.5KB is
        contiguous in DRAM (large descriptors -> full HBM bandwidth).
"""

import os
import sys

for _p in ("/opt/trn_rl_repo", "/root/.axon_site/_ro/trn_rl_repo"):
    if os.path.isdir(_p) and _p not in sys.path:
        sys.path.append(_p)

import numpy as np

import concourse.bass as bass
import concourse.tile as tile
from concourse import bacc, mybir
from concourse.bass_utils import run_bass_kernel_spmd

# Problem dims (hardcoded per contract)
B, T, E = 8, 1000, 512
U = 100
U1 = U + 1          # 101 joint positions
H, D, P = 2, 256, 256
J, V = 128, 88
BLANK = V - 1
N_CORES = 8

# Device tiling
TP = 1024           # padded T (8 blocks of 128)
TB = 128            # t-steps per block (= joint-matmul lhsT cols, FWL)
NB = TP // TB       # 8 blocks
HALF = 8            # t-steps per A-PSUM tile ([128, 1024] = 2 banks)
CH = 4              # t-steps per pre-add matmul chunk (N = 404)
SPAN = 16           # t-steps per combined lhsT tile (K = SPAN + U1 = 117)
UG = 10             # u-steps per M-PSUM tile ([128, 1024] = 2 banks)
NUG = 11            # u-groups per block (10 full + 1 leftover)

F32 = mybir.dt.float32
F16 = mybir.dt.float16

_CACHE = {}


def _build_program(reps=1):
    nc = bacc.Bacc("TRN2", target_bir_lowering=False, debug=False)

    encT = nc.dram_tensor("encT", [E, TP], F16, kind="ExternalInput").ap()
    jw1enc = nc.dram_tensor("jw1enc", [E, J], F16, kind="ExternalInput").ap()
    jw2d = nc.dram_tensor("jw2d", [J, V], F16, kind="ExternalInput").ap()
    biasu = nc.dram_tensor("biasu", [U1, J], F16, kind="ExternalInput").ap()
    seld = nc.dram_tensor("seld", [SPAN + U1, SPAN * U1], F16, kind="ExternalInput").ap()
    out = nc.dram_tensor("out", [T, U1 * V], F32, kind="ExternalOutput").ap()

    with tile.TileContext(nc) as tc:
        with (
            tc.tile_pool(name="singles", bufs=1) as singles,
            tc.tile_pool(name="hidp", bufs=3) as hidp,
            tc.tile_pool(name="stgp", bufs=2) as stgp,
            tc.tile_pool(name="psA", bufs=2, space="PSUM") as psA,
            tc.tile_pool(name="psM", bufs=2, space="PSUM") as psM,
        ):
            # ---- persistent SBUF tensors ----
            encT_sb = []
            for k in range(4):
                t_ = singles.tile([128, TP], F16, tag=f"encT{k}")
                nc.sync.dma_start(out=t_[:, :], in_=encT[k * 128:(k + 1) * 128, :])
                encT_sb.append(t_)
            jw1_sb = []
            for k in range(4):
                t_ = singles.tile([128, J], F16, tag=f"jw1_{k}")
                nc.sync.dma_start(out=t_[:, :], in_=jw1enc[k * 128:(k + 1) * 128, :])
                jw1_sb.append(t_)
            jw2_sb = singles.tile([J, V], F16, tag="jw2")
            nc.sync.dma_start(out=jw2_sb[:, :], in_=jw2d[:, :])
            sel_sb = singles.tile([SPAN + U1, SPAN * U1], F16, tag="sel")
            nc.sync.dma_start(out=sel_sb[:, :], in_=seld[:, :])
            # two alternating combined lhsT tiles; bias rows are static
            combined = []
            for i in range(2):
                t_ = singles.tile([SPAN + U1, J], F16, tag=f"comb{i}")
                nc.sync.dma_start(out=t_[SPAN:SPAN + U1, :], in_=biasu[:, :])
                combined.append(t_)
            # enc_proj row-major: [t % 128, (t//128)*J + j] (block-aligned)
            enc_proj = singles.tile([128, NB * J], F16, tag="encproj")

            for rep in range(reps):
                _emit_rep(nc, hidp, stgp, psA, psM,
                          encT_sb, jw1_sb, jw2_sb, sel_sb, combined, enc_proj,
                          out, rep)

    nc.compile()
    return nc


def _emit_rep(nc, hidp, stgp, psA, psM,
              encT_sb, jw1_sb, jw2_sb, sel_sb, combined, enc_proj, out, rep):
    # ---- prologue: enc_proj[t, j] = sum_e enc[t, e] * jw1enc[e, j] ----
    for cb in range(NB):
        ep = psA.tile([TB, J], F32, tag="A", name=f"ep{rep}_{cb}")
        for k in range(4):
            nc.tensor.matmul(
                ep[:, :],
                encT_sb[k][:, cb * TB:(cb + 1) * TB],
                jw1_sb[k][:, :],
                start=(k == 0),
                stop=(k == 3),
            )
        nc.vector.tensor_copy(out=enc_proj[:, cb * J:(cb + 1) * J], in_=ep[:, :])

    hid_tiles = [None] * NB
    stg_tiles = [None] * NB

    def front(b, step):
        # pre-add matmuls + tanh(+transpose) for half (b, step)
        t0 = b * TB + step * HALF
        if step == 0:
            hid_tiles[b] = hidp.tile([128, U1 * TB], F16, tag="hid",
                                     name=f"hid{rep}_{b}")
        hid2 = hid_tiles[b]
        span_i = t0 // SPAN
        cb_t = combined[span_i % 2]
        if t0 % SPAN == 0:
            # load this span's 16 enc_proj rows into the combined tile
            blk, p0 = t0 // TB, t0 % TB
            nc.gpsimd.dma_start(
                out=cb_t[0:SPAN, :],
                in_=enc_proj[p0:p0 + SPAN, blk * J:(blk + 1) * J],
            )
        A = psA.tile([128, 1024], F32, tag="A", name=f"A{rep}_{b}_{step}")
        lh = (t0 % SPAN) // HALF  # 0 or 1: which pair of chunks in sel
        for c in range(2):
            nc.tensor.matmul(
                A[:, c * 512:c * 512 + CH * U1],
                cb_t[:, :],
                sel_sb[:, (lh * 2 + c) * CH * U1:(lh * 2 + c + 1) * CH * U1],
                start=True,
                stop=True,
            )
        # tanh, contiguous t-major write; the joint matmul reads strided
        base = step * HALF
        nc.scalar.activation(
            out=hid2[:, base * U1:(base + HALF) * U1].rearrange(
                "p (c x) -> p c x", c=2),
            in_=A.rearrange("p (c x) -> p c x", c=2)[:, :, 0:CH * U1],
            func=mybir.ActivationFunctionType.Tanh,
        )

    def back(b, ug):
        # joint matmuls + evacuation for u-group ug of block b
        hid2 = hid_tiles[b]
        if ug == 0:
            stg_tiles[b] = stgp.tile([TB, U1 * V], F32, tag="stg",
                                     name=f"stg{rep}_{b}")
        stg = stg_tiles[b]
        u0 = ug * UG
        n_u = UG if ug < NUG - 1 else U1 - u0
        M = psM.tile([TB, 1024], F32, tag="M", name=f"M{rep}_{b}_{ug}")
        hid_ut = hid2.rearrange("p (t u) -> p u t", u=U1)  # [128, 101, 128] strided
        for i in range(n_u):
            col = (i // 5) * 512 + (i % 5) * V
            nc.tensor.matmul(
                M[:, col:col + V],
                hid_ut[:, u0 + i, :],
                jw2_sb[:, :],
                start=True,
                stop=True,
            )
        if n_u == UG:
            nc.vector.tensor_copy(
                out=stg[:, u0 * V:(u0 + UG) * V].rearrange("p (bk x) -> p bk x", bk=2),
                in_=M.rearrange("p (bk x) -> p bk x", bk=2)[:, :, 0:5 * V],
            )
        else:
            nc.vector.tensor_copy(
                out=stg[:, u0 * V:(u0 + n_u) * V],
                in_=M[:, 0:n_u * V],
            )
        if ug == NUG - 1:
            n_t = min(TB, T - b * TB)
            nc.sync.dma_start(
                out=out[b * TB:b * TB + n_t, :],
                in_=stg[0:n_t, :],
            )

    # software-pipelined emission: block b's fronts interleave with b-1's backs
    for b in range(NB):
        for step in range(TB // HALF):  # 16
            front(b, step)
            if b >= 1 and step < NUG:
                back(b - 1, step)
    for ug in range(NUG):
        back(NB - 1, ug)


def _host_pred_bias(targets_b, emb, pw1, pb1, pw2, pb2, jw1, jb1):
    """bias[u, j] = (pred @ jw1[E:] + jb1)[u, j] for the 101 joint positions."""
    ext = np.concatenate([np.full(H, BLANK, np.int64), targets_b.astype(np.int64)])
    e = np.concatenate([emb[ext[1:U1 + 1]], emb[ext[0:U1]]], axis=1)  # [101, 512]
    h = np.tanh(e @ pw1 + pb1)
    pred = np.tanh(h @ pw2 + pb2)
    return (pred @ jw1[E:] + jb1).astype(np.float32)  # [101, 128]


def _make_sel():
    sel = np.zeros((SPAN + U1, SPAN * U1), np.float16)
    for tl in range(SPAN):
        sel[tl, tl * U1:(tl + 1) * U1] = 1.0
        sel[SPAN:SPAN + U1, tl * U1:(tl + 1) * U1] += np.eye(U1, dtype=np.float16)
    return sel


def _make_in_maps(encoder_states, targets, emb, pw1, pb1, pw2, pb2, jw1, jb1, jw2):
    encoder_states = np.asarray(encoder_states, dtype=np.float32)
    jw1 = np.asarray(jw1, dtype=np.float32)
    jw2_np = np.ascontiguousarray(np.asarray(jw2, dtype=np.float32)).astype(np.float16)
    jw1enc = np.ascontiguousarray(jw1[:E]).astype(np.float16)
    sel = _make_sel()

    in_maps = []
    for b in range(B):
        encT_b = np.zeros((E, TP), np.float16)
        encT_b[:, :T] = encoder_states[b].T.astype(np.float16)
        bias_b = _host_pred_bias(
            np.asarray(targets[b]), np.asarray(emb, np.float32),
            np.asarray(pw1, np.float32), np.asarray(pb1, np.float32),
            np.asarray(pw2, np.float32), np.asarray(pb2, np.float32),
            jw1, np.asarray(jb1, np.float32),
        ).astype(np.float16)
        in_maps.append({
            "encT": encT_b,
            "jw1enc": jw1enc,
            "jw2d": jw2_np,
            "biasu": bias_b,
            "seld": sel,
        })
    return in_maps


def kernel(encoder_states, encoder_states_size, targets, targets_size,
           emb, pw1, pb1, pw2, pb2, jw1, jb1, jw2, jb2):
    if "nc" not in _CACHE:
        _CACHE["nc"] = _build_program()
    nc = _CACHE["nc"]

    in_maps = _make_in_maps(encoder_states, targets, emb, pw1, pb1, pw2, pb2,
                            jw1, jb1, jw2)
    _CACHE["in_maps"] = in_maps
    res = run_bass_kernel_spmd(nc, in_maps, core_ids=list(range(N_CORES)))

    out = np.empty((B, T, U1, V), np.float32)
    for b in range(B):
        out[b] = res.results[b]["out"].reshape(T, U1, V)
    out += np.asarray(jb2, np.float32)  # jb2 epilogue (host)
    # ragged masking (host epilogue)
    tsz = np.asarray(encoder_states_size).astype(np.int64)
    usz = np.asarray(targets_size).astype(np.int64) + 1
    for b in range(B):
        out[b, tsz[b]:, :, :] = 0.0
        out[b, :, usz[b]:, :] = 0.0
    return out
